# revision 7
# baseline (speedup 1.0000x reference)
"""Trainium2 Bass kernel for BinaryLinear: out = x @ sign(W).T

Shapes (hardcoded): x [32768, 2048] f32, weight [2048, 2048] f32,
out [32768, 2048] f32.

Strategy: data-parallel over 8 NeuronCores — shard the token axis
(4096 tokens/core), replicate the weight. Per core:
  - x is loaded with casting SWDGE DMAs (f32 in HBM -> bf16 in SBUF)
    on the GpSimd queue, then transposed tile-wise with batched xbar
    DMA-transposes -> xT[i, ic, t].
  - weight prep: f32 loads on the Scalar HWDGE queue interleaved with
    Sign activations (f32 -> bf16), then batched xbar DMA-transposes
    into the resident swT[i, ot, ic, o] (contiguous per-ot writes).
  - matmuls run in 8-token-tile blocks x 4 output-chunk waves so the
    first waves depend only on the first weight tiles: out[128t, 512o]
    accumulates 16 bf16 matmuls (xT chunk stationary, swT moving) in
    PSUM, DVE copies PSUM->SBUF, per-chunk DMA stores.

Engine roles: GpSimd = x cast-loads; Scalar = w loads + Sign; Sync =
all xbar transposes (single engine — concurrent DMA_TRANSPOSE from two
engines corrupts the shared xbar) + output stores; Vector = PSUM
copies; Tensor = the 2048 N=512 matmuls (~213 ns each warm).
"""

import sys

if "/opt/trn_rl_repo" not in sys.path:
    sys.path.insert(0, "/opt/trn_rl_repo")

import numpy as np

T, I, O = 32768, 2048, 2048
NCORES = 8
TL = T // NCORES  # tokens per core

_NC = None


def _build():
    import concourse.bacc as bacc
    import concourse.mybir as mybir
    from concourse import tile
    from contextlib import ExitStack

    f32 = mybir.dt.float32
    bf16 = mybir.dt.bfloat16

    IC = I // 128  # i-chunks (contraction)
    OT = O // 128  # weight row tiles
    NT = TL // 128  # token tiles per core
    OCW = 512  # matmul moving free dim
    NOC = O // OCW
    BLK = 8  # token tiles per block
    NBLK = NT // BLK

    nc = bacc.Bacc("TRN2", target_bir_lowering=False, debug=False, num_devices=NCORES)
    x = nc.dram_tensor("x", [TL, I], f32, kind="ExternalInput")
    w = nc.dram_tensor("weight", [O, I], f32, kind="ExternalInput")
    out = nc.dram_tensor("out", [TL, O], f32, kind="ExternalOutput")

    with tile.TileContext(nc) as tc, ExitStack() as ctx:
        # sign(W).T resident: swT[i_p, ot, ic, o_l] =
        # sign(W)[128*ot + o_l, 128*ic + i_p]
        swt_pool = ctx.enter_context(tc.tile_pool(name="swt", bufs=1))
        swT = swt_pool.tile([128, OT, IC, 128], bf16)

        wprep = ctx.enter_context(tc.tile_pool(name="wprep", bufs=1))
        w_f32 = [
            wprep.tile([128, I], f32, tag="w_f32", name=f"w_f32_{ot}", bufs=4)
            for ot in range(OT)
        ]
        w_sgn = [
            wprep.tile([128, I], bf16, tag="w_sgn", name=f"w_sgn_{ot}", bufs=3)
            for ot in range(OT)
        ]
        # interleave loads and signs so the in-order Scalar stream never
        # waits on a pool slot whose release is behind it
        for ot in range(4):
            nc.scalar.dma_start(w_f32[ot][:], w[128 * ot : 128 * (ot + 1), :])
        for ot in range(OT):
            nc.scalar.activation(
                w_sgn[ot][:], w_f32[ot][:], mybir.ActivationFunctionType.Sign
            )
            if ot + 4 < OT:
                nc.scalar.dma_start(
                    w_f32[ot + 4][:], w[128 * (ot + 4) : 128 * (ot + 5), :]
                )

        xpool = ctx.enter_context(tc.tile_pool(name="xpool", bufs=4))
        xtpool = ctx.enter_context(tc.tile_pool(name="xtpool", bufs=10))
        opool = ctx.enter_context(tc.tile_pool(name="opool", bufs=12))
        psum_mm = ctx.enter_context(tc.tile_pool(name="psum_mm", bufs=4, space="PSUM"))

        # all x tiles: casting DMA loads on the GpSimd (SWDGE) queue
        x_bf = []
        for tt in range(NT):
            xb = xpool.tile([128, I], bf16, tag="x_bf", name=f"x_bf_{tt}")
            nc.gpsimd.dma_start(xb[:], x[128 * tt : 128 * (tt + 1), :])
            x_bf.append(xb)

        xT = [None] * NT
        for blk in range(NBLK):
            tts = range(blk * BLK, (blk + 1) * BLK)
            if blk == 0:
                # first 4 weight transposes ahead of the first xTs so
                # wave 0 can start as soon as xT[0] lands
                for ot in range(4):
                    nc.sync.dma_start(swT[:, ot, :, :], w_sgn[ot][:], transpose=True)
            for tt in tts:
                xT[tt] = xtpool.tile([128, IC, 128], bf16, tag="xT", name=f"xT_{tt}")
                nc.sync.dma_start(xT[tt][:], x_bf[tt][:], transpose=True)
            if blk == 0:
                for ot in range(4, OT):
                    nc.sync.dma_start(swT[:, ot, :, :], w_sgn[ot][:], transpose=True)

            for oc in range(NOC):
                for tt in tts:
                    acc = psum_mm.tile(
                        [128, OCW], f32, tag="acc", name=f"acc_{tt}_{oc}"
                    )
                    for ic in range(IC):
                        nc.tensor.matmul(
                            acc[:],
                            xT[tt][:, ic, :],
                            swT[:, 4 * oc : 4 * (oc + 1), ic, :],
                            start=(ic == 0),
                            stop=(ic == IC - 1),
                        )
                    o_ch = opool.tile([128, OCW], f32, tag="o_ch")
                    nc.vector.tensor_copy(o_ch[:], acc[:])
                    nc.sync.dma_start(
                        out[128 * tt : 128 * (tt + 1), OCW * oc : OCW * (oc + 1)],
                        o_ch[:],
                    )

    nc.compile()
    return nc


def _get_nc():
    global _NC
    if _NC is None:
        _NC = _build()
    return _NC


def _in_maps(x, w):
    x = np.ascontiguousarray(np.asarray(x, dtype=np.float32))
    w = np.ascontiguousarray(np.asarray(w, dtype=np.float32))
    assert x.shape == (T, I) and w.shape == (O, I)
    return [
        {"x": x[c * TL : (c + 1) * TL], "weight": w} for c in range(NCORES)
    ]


def kernel(**inputs):
    from concourse.bass_utils import run_bass_kernel_spmd

    nc = _get_nc()
    res = run_bass_kernel_spmd(
        nc, _in_maps(inputs["x"], inputs["weight"]), core_ids=list(range(NCORES))
    )
    return np.concatenate([r["out"] for r in res.results], axis=0)
